# revision 27
# baseline (speedup 1.0000x reference)
"""Trainium2 Bass kernel for the DigitalTwinModel (3-layer LSTM digital twin).

Strategy: 4-way model parallelism (hidden dim) x 2-way data parallelism
(batch), with the per-timestep h-slice AllGather implemented as direct
SBUF->SBUF remote DMA (remote_dma_broadcast) instead of ncfw collectives.

  - The 8 NeuronCores form two XOR-closed exchange groups dictated by the
    physical fabric: logical cores {0,1,6,7} (batch rows 0:128) and
    {2,3,4,5} (rows 128:256).  Probing tpb_base shows logical k sits at
    physical TPB [2,3,6,7]/[6,7,2,3]; within a group the logical XOR
    deltas {1,6,7} map to physical TPB deltas {1,4,5} (Delta-1 intra-chip
    RMTV, Delta-4/5 cross-die D2D, which must ride rdests slots 4-7).
  - Within a group, core k owns hidden features sl(k)*256:(sl(k)+1)*256 of
    every LSTM layer's h/c state (sl = rank within the sorted group) and
    the matching 4*256 gate rows of W_ih/W_hh.
  - Per timestep, 3 exchanges (one per layer's h slice) replace the old
    3 AllGathers: each core fires 3 relative-dest remote_dma_broadcasts
    (64KB each, disjoint DMA-engine slots) straight from the SBUF h tile
    into the 3 peers' SBUF landing tiles -- no DRAM staging, no readback,
    ~2us instead of ~21.5us per round.  Descriptor generation is hoisted
    off the chain: tile_critical lazy entry lets Pool desc-gen run during
    the producing compute, and the arrival waits execute on the idle SP
    engine so Pool rolls straight into the next exchange's descriptors.
  - Arrival sync: 3 plain semaphores, one per delta slot, bumped +2 per
    delivery, waited at static cumulative thresholds; per-slot waits make
    the pacing airtight (a fast peer one round ahead cannot satisfy a
    slow slot's wait, and per-link deliveries are FIFO).  A one-time
    8-core AllReduce barrier precedes the first send so no core fires
    into a peer that has not cleared its semaphores yet.  Sender-side
    buffer reuse is guarded by the deferred local_sem wait (one exchange
    late, off the critical path).
  - Landing tiles are double-buffered per (layer, slot) on timestep
    parity; the LSTM dependence chain proves peers cannot overwrite a
    parity buffer before its two readers (W_ih of the next layer, W_hh of
    the next step) are done.
  - Consumers read gathered h as 4 k-blocks: block 0 is the local h tile,
    blocks 1-3 the landing slots; the per-core XOR block order is folded
    into the host-side column permutation of W_hh/W_ih(1,2)/Wd1.
  - All matmul operands are bf16, PSUM accumulation and persistent cell
    state fp32.  Biases fold into PSUM as 1-row matmuls; decoder algebra
    enc = relu(M @ r + cm) with M = We @ Wd2 keeps the output projection
    off the chain; out = Wd2 @ r is column-sharded 4 ways (each core
    writes its 128 of 512 output columns; bd2 added on the host).
  - TimelineSim cannot model remote-DMA data movement in no_exec mode;
    test.py prices THIS program with a cost model subclass that appends
    the D2D transfer occupancy + ack and the semaphore deliveries at each
    trigger_dma (see test.py).  build_program(comm="stub") additionally
    provides a local-DMA twin used only for debugging.
"""

import numpy as np
import ml_dtypes

import concourse.bass as bass
import concourse.mybir as mybir
from concourse import bacc
import concourse.tile as tile
from concourse.bass_utils import run_bass_kernel_spmd

F32 = mybir.dt.float32
BF16 = mybir.dt.bfloat16
U32 = mybir.dt.uint32
AF = mybir.ActivationFunctionType

B, D_IN, H, L, T = 256, 512, 1024, 3, 32
NCORES = 8
P = 128
MP = 4                    # model-parallel ways (hidden shard) per group
DP = 2                    # data-parallel groups
SH = H // MP              # 256 hidden features owned per core per layer
NPT = SH // P             # 2 partition-tiles per owned slice
BG = B // DP              # 128 batch rows per group
KT_H = H // P             # 8 k-tiles over hidden dim
MT_G = 4 * SH // P        # 8 gate m-tiles per core (gate-major: i,i,g,g,f,f,o,o)

# XOR-closed exchange groups (physical fabric; see module docstring)
GROUPS = [[0, 1, 6, 7], [2, 3, 4, 5]]
GROUP_OF = {k: g for g, grp in enumerate(GROUPS) for k in grp}
SL = {k: j for grp in GROUPS for j, k in enumerate(grp)}  # member rank
DLOG = (1, 6, 7)          # logical XOR deltas, slot order
PDELT = (1, 4, 5)         # physical TPB deltas, slot order
PSLOT = {1: 0, 4: 4, 5: 5}  # rdests slot per physical delta (D2D in 4-7)
N_WARM1 = [0, 0, 0]       # PE warmers at exchange launch (per slot)
N_WARM2 = [0, 0, 0]       # PE warmers after arrival wait (per slot)


def _pe_touch(nc, ap2d):
    """Tiny ldweights that makes the PE observe a tile's producer semaphore."""
    nc.tensor.ldweights(weights=ap2d[0:1, 0:2].bitcast(BF16))


def build_program(timesteps=T, comm="rdma"):
    nc = bacc.Bacc(None, num_devices=NCORES, dynamic_dma_scratch_size=16384,
                   monotonic_sem_count=3, num_swdge_queues=1)

    # ---- kernel I/O (per-core payloads supplied from the host) ----
    wih = [nc.dram_tensor(f"wih{l}", [H, 4 * SH], BF16, kind="ExternalInput") for l in range(L)]
    whh = [nc.dram_tensor(f"whh{l}", [H, 4 * SH], BF16, kind="ExternalInput") for l in range(L)]
    bgr = [nc.dram_tensor(f"bg{l}", [1, 4 * SH], BF16, kind="ExternalInput") for l in range(L)]
    wd1 = nc.dram_tensor("wd1", [H, H], BF16, kind="ExternalInput")
    mmat = nc.dram_tensor("mmat", [H, H], BF16, kind="ExternalInput")
    wd2 = nc.dram_tensor("wd2", [H, P], BF16, kind="ExternalInput")
    bd1r = nc.dram_tensor("bd1r", [1, H], BF16, kind="ExternalInput")
    cmr = nc.dram_tensor("cmr", [1, H], BF16, kind="ExternalInput")
    enc0 = nc.dram_tensor("enc0", [H, BG], BF16, kind="ExternalInput")
    # output sharded over the group: each core writes its 128 of 512 columns
    out = nc.dram_tensor("out", [BG, timesteps, P], F32, kind="ExternalOutput")

    # per-delta-slot arrival semaphores (+2 per delivery); static cumulative
    # thresholds (no control flow) keep them priceable in no_exec simulation
    slot_sems = [nc.alloc_semaphore(f"rdma_slot{j}") for j in range(3)]
    local_sem = nc.alloc_semaphore("rdma_local")
    prep_sems = [nc.alloc_semaphore(f"rdma_prep{q}") for q in range(3)]
    nc._rdma_meta = {"slot_sems": slot_sems, "local_sem": local_sem}

    with tile.TileContext(nc) as tc:
        with (
            tc.tile_pool(name="singles", bufs=1) as singles,
            tc.tile_pool(name="encp", bufs=2) as encp,
            tc.tile_pool(name="rtp", bufs=2) as rtp,
            tc.tile_pool(name="gtmp", bufs=2) as gtmp,
            tc.tile_pool(name="hloc", bufs=2) as hloc,
            tc.tile_pool(name="obp", bufs=2) as obp,
            tc.tile_pool(name="pgp", bufs=1, space="PSUM") as pgp,
            tc.tile_pool(name="pwork", bufs=2, space="PSUM") as pwork,
            tc.tile_pool(name="poutp", bufs=2, space="PSUM") as poutp,
            tc.tile_pool(name="pwarm", bufs=1, space="PSUM") as pwarm,
            tc.tile_pool(name="dram", bufs=1, space="DRAM") as dram,
        ):
            # ---- startup barrier: AllReduce over all 8 cores ----
            bar_in = dram.tile([1, 16], F32, tag="bar_in", name="bar_in")
            bar_out = dram.tile([1, 16], F32, tag="bar_out", name="bar_out")
            barT = singles.tile([1, 16], F32, tag="barT", name="barT")
            nc.vector.memset(barT, 1.0)
            nc.gpsimd.dma_start(out=bar_in, in_=barT)
            nc.gpsimd.collective_compute(
                "AllReduce", mybir.AluOpType.add,
                replica_groups=[list(range(NCORES))],
                ins=[bar_in.opt()], outs=[bar_out.opt()])
            barS = singles.tile([1, 16], F32, tag="barS", name="barS")
            nc.gpsimd.dma_start(out=barS, in_=bar_out)

            # ---- load resident weights/biases into SBUF ----
            encT = encp.tile([P, KT_H, BG], BF16, tag="enc", name="enc")
            nc.sync.dma_start(out=encT, in_=enc0[:].rearrange("(kk p) b -> p kk b", p=P))
            s_bg = []
            t_ = singles.tile([1, 4 * SH], BF16, tag="sbg0", name="sbg0")
            nc.sync.dma_start(out=t_, in_=bgr[0][:])
            _pe_touch(nc, t_)
            s_bg.append(t_)
            s_wih, s_whh = [], []
            for l in range(L):
                w = singles.tile([P, KT_H, 4 * SH], BF16, tag=f"swih{l}", name=f"swih{l}")
                nc.sync.dma_start(out=w, in_=wih[l][:].rearrange("(kk p) m -> p kk m", p=P))
                _pe_touch(nc, w[:, 0, :])
                s_wih.append(w)
            for l in range(L):
                w = singles.tile([P, KT_H, 4 * SH], BF16, tag=f"swhh{l}", name=f"swhh{l}")
                nc.sync.dma_start(out=w, in_=whh[l][:].rearrange("(kk p) m -> p kk m", p=P))
                _pe_touch(nc, w[:, 0, :])
                s_whh.append(w)
            for l in range(1, L):
                t_ = singles.tile([1, 4 * SH], BF16, tag=f"sbg{l}", name=f"sbg{l}")
                nc.sync.dma_start(out=t_, in_=bgr[l][:])
                _pe_touch(nc, t_)
                s_bg.append(t_)
            s_bd1 = singles.tile([1, H], BF16, tag="sbd1", name="sbd1")
            nc.sync.dma_start(out=s_bd1, in_=bd1r[:])
            _pe_touch(nc, s_bd1)
            s_cm = singles.tile([1, H], BF16, tag="scm", name="scm")
            nc.sync.dma_start(out=s_cm, in_=cmr[:])
            _pe_touch(nc, s_cm)
            s_wd1 = singles.tile([P, KT_H, H], BF16, tag="swd1", name="swd1")
            nc.sync.dma_start(out=s_wd1, in_=wd1[:].rearrange("(kk p) m -> p kk m", p=P))
            _pe_touch(nc, s_wd1[:, 0, :])
            s_mm = singles.tile([P, KT_H, H], BF16, tag="smm", name="smm")
            nc.sync.dma_start(out=s_mm, in_=mmat[:].rearrange("(kk p) m -> p kk m", p=P))
            _pe_touch(nc, s_mm[:, 0, :])
            s_wd2 = singles.tile([P, KT_H, P], BF16, tag="swd2", name="swd2")
            nc.sync.dma_start(out=s_wd2, in_=wd2[:].rearrange("(kk p) m -> p kk m", p=P))
            _pe_touch(nc, s_wd2[:, 0, :])
            ones = singles.tile([1, BG], BF16, tag="ones", name="ones")
            nc.vector.memset(ones, 1.0)
            _pe_touch(nc, ones)

            # persistent cell state (zero-initialised), fp32
            s_c = []
            for l in range(L):
                c = singles.tile([P, NPT, BG], F32, tag=f"c{l}", name=f"c{l}")
                nc.vector.memset(c, 0.0)
                s_c.append(c)

            # landing tiles: [channel][parity][slot]; channels 0-2 are
            # the LSTM layers' h, 3 is the decoder r, 4 the re-encoding
            land = [[[singles.tile([P, NPT, BG], BF16, tag=f"ld{l}_{p}_{j}",
                                   name=f"ld{l}_{p}_{j}")
                      for j in range(3)] for p in range(2)] for l in range(L)]

            # PE warmer scratch (never read)
            warm_ps = pwarm.tile([P, 512], F32, tag="warm", name="warm")

            def warm(n):
                for _ in range(n):
                    nc.tensor.matmul(
                        warm_ps[:, 0:256], lhsT=s_wd1[:, 0, 0:P], rhs=s_wd1[:, 0, 0:256],
                        start=True, stop=True)

            xcnt = [0]

            def xchg(chan, t, src):
                """Send my slice to the 3 group peers, SBUF->SBUF.

                Descriptor-gen (Pool) is hoisted by the lazy crit entry and
                by keeping Pool free of arrival waits: the per-slot arrival
                waits run on SP, so Pool rolls straight into the next
                exchange's desc-gen while this one is still in flight."""
                par = t % 2
                n = xcnt[0]
                xcnt[0] += 1
                if comm == "stub":
                    for j in range(3):
                        nc.gpsimd.dma_start(out=land[chan][par][j], in_=src)
                    sc = gtmp.tile([P, 8], BF16, tag="xsc", name="xsc")
                    nc.gpsimd.dma_start(out=sc, in_=src[:, 0, 0:8])
                    return
                with tc.tile_critical(no_gpsimd_drain=True):
                    g = nc.gpsimd
                    for j, d in enumerate(PDELT):
                        rdests = [None] * 8
                        rdests[PSLOT[d]] = (0, d)
                        g.remote_dma_broadcast(
                            out_ap=land[chan][par][j][:],
                            in_ap=src[:],
                            remote_sem=slot_sems[j],
                            local_sem=local_sem,
                            rdests=rdests,
                        ).then_inc(prep_sems[0], 1)
                    # lazy crit entry: desc-gen above runs concurrently with
                    # the producer of src; data/ctrl waits attach here.
                    tc.wait_critical_data_deps()
                    if n == 0:
                        breg = g.alloc_register("bgate")
                        g.reg_load(breg, barS[0:1, 0:1].bitcast(U32))
                        g.free_register(breg)
                    else:
                        g.wait_ge(local_sem, 48 * n)  # round n-1 sends drained
                    g.wait_ge(prep_sems[0], 3 * (n + 1))
                    g.trigger_dma(count=3)
                    for j in range(3):
                        nc.sync.wait_ge(slot_sems[j], 2 * (n + 1))  # arrivals (SP)

            def mix_rhs(hl_t, lands_t):
                """Gathered-h rhs: block 0 local, blocks 1-3 landed slots."""
                def rhs(kk):
                    b, i = kk // NPT, kk % NPT
                    src = hl_t if b == 0 else lands_t[b - 1]
                    return src[:, i, :]
                return rhs

            def nat_rhs(x):
                return lambda kk: x[:, kk, :]

            # Gates in THREE PSUM tiles closed independently; host column
            # order is i, g, f, o (pgA=[i,g] closes first).
            GATE_GROUPS = ((0, 2 * NPT), (2 * NPT, 3 * NPT), (3 * NPT, MT_G))

            def gate_mms(pg3, w, rhs_kk, last):
                for pg_t, (m0, m1) in zip(pg3, GATE_GROUPS):
                    for kk in range(KT_H):
                        rhs = rhs_kk(kk)
                        for m in range(m0, m1):
                            nc.tensor.matmul(
                                pg_t[:, m - m0, :],
                                lhsT=w[:, kk, m * P:(m + 1) * P],
                                rhs=rhs,
                                start=False,
                                stop=(last and kk == KT_H - 1 and m == m1 - 1),
                            )

            def preissue(l, t_eff, rhs_prev):
                """Open this stage's PSUM banks with bias, add W_hh part."""
                pg3 = (
                    pgp.tile([P, 2 * NPT, BG], F32, tag="pgA", name="pgA"),
                    pgp.tile([P, NPT, BG], F32, tag="pgF", name="pgF"),
                    pgp.tile([P, NPT, BG], F32, tag="pgO", name="pgO"),
                )
                for pg_t, (m0, m1) in zip(pg3, GATE_GROUPS):
                    for m in range(m0, m1):
                        nc.tensor.matmul(
                            pg_t[:, m - m0, :],
                            lhsT=s_bg[l][:, m * P:(m + 1) * P],
                            rhs=ones,
                            start=(m == m0),
                            stop=False,
                        )
                if t_eff > 0:
                    gate_mms(pg3, s_whh[l], rhs_prev, last=False)
                return pg3

            def ew(l, pg2, first_step, hl):
                """gates -> h'_slice bf16 into hl, update fp32 c in place."""
                pgA, pgF, pgO = pg2
                gi = pgA[:, 0 * NPT:1 * NPT, :]
                gg = pgA[:, 1 * NPT:2 * NPT, :]
                gf = pgF[:, :, :]
                go = pgO[:, :, :]
                cc = s_c[l]
                ti = gtmp.tile([P, NPT, BG], F32, tag="ti", name="ti")
                tg = gtmp.tile([P, NPT, BG], F32, tag="tg", name="tg")
                to = gtmp.tile([P, NPT, BG], F32, tag="to", name="to")
                nc.scalar.activation(ti, gi, AF.Sigmoid)
                nc.scalar.activation(tg, gg, AF.Tanh)
                if first_step:
                    nc.scalar.activation(to, go, AF.Sigmoid)
                    nc.vector.tensor_mul(cc, ti, tg)   # c = i*g
                else:
                    tf = gtmp.tile([P, NPT, BG], F32, tag="tf", name="tf")
                    t1 = gtmp.tile([P, NPT, BG], F32, tag="t1", name="t1")
                    t2 = gtmp.tile([P, NPT, BG], F32, tag="t2", name="t2")
                    nc.vector.tensor_mul(t1, ti, tg)       # i * g
                    nc.scalar.activation(tf, gf, AF.Sigmoid)
                    nc.vector.tensor_mul(t2, tf, cc)       # f * c
                    nc.scalar.activation(to, go, AF.Sigmoid)
                    nc.vector.tensor_add(cc, t1, t2)
                tanhc = gtmp.tile([P, NPT, BG], F32, tag="tg", name="tg")  # tg dead
                nc.scalar.activation(tanhc, cc, AF.Tanh)
                nc.vector.tensor_mul(hl, to, tanhc)
                return hl

            def new_hl(l):
                return hloc.tile([P, NPT, BG], BF16, tag=f"hl{l}", name=f"hl{l}")

            def dec_pair(wtile, rhs_kk, brow, dst):
                """dst[:, 2m:2m+2, :] = relu(w^T @ x + b) with paired-m PSUM."""
                for mp2 in range(KT_H // 2):
                    pd = pwork.tile([P, 2, BG], F32, tag="pd", name="pd")
                    for j in range(2):
                        m = 2 * mp2 + j
                        nc.tensor.matmul(
                            pd[:, j, :], lhsT=brow[:, m * P:(m + 1) * P], rhs=ones,
                            start=(j == 0), stop=False)
                    for kk in range(KT_H):
                        rhs = rhs_kk(kk)
                        for j in range(2):
                            m = 2 * mp2 + j
                            nc.tensor.matmul(
                                pd[:, j, :],
                                lhsT=wtile[:, kk, m * P:(m + 1) * P],
                                rhs=rhs,
                                start=False,
                                stop=(kk == KT_H - 1 and j == 1),
                            )
                    nc.scalar.activation(dst[:, 2 * mp2:2 * mp2 + 2, :], pd, AF.Relu)

            def outwrite(tstep, r_kk):
                """out[:, t, :] = (r^T @ Wd2^T) own column slice; bd2 on host."""
                po = poutp.tile([BG, P], F32, tag="po", name="po")
                for kk in range(KT_H):
                    nc.tensor.matmul(
                        po,
                        lhsT=r_kk(kk),
                        rhs=s_wd2[:, kk, :],
                        start=kk == 0,
                        stop=kk == KT_H - 1,
                    )
                ob = obp.tile([BG, P], F32, tag="ob", name="ob")
                nc.vector.tensor_copy(out=ob, in_=po)
                nc.sync.dma_start(out=out[:, tstep, :], in_=ob)

            # ---- prologue: L0(0) before the first exchange ----
            pg = preissue(0, 0, None)
            _pe_touch(nc, encT[:, 0, :])
            gate_mms(pg, s_wih[0], nat_rhs(encT), last=True)
            hl0 = ew(0, pg, True, new_hl(0))
            hlp = [None] * L          # previous-step local h per layer

            r_mix_prev = None
            for t in range(timesteps):
                par, prv = t % 2, (t - 1) % 2
                last_step = t == timesteps - 1

                # ---- slot 0: exchange h0(t); window: L1(t) ----
                xchg(0, t, hl0)
                pg = preissue(1, t, None if t == 0 else mix_rhs(hlp[1], land[1][prv]))
                if r_mix_prev is not None:
                    outwrite(t - 1, r_mix_prev)   # deferred, r(t-1) fully landed
                gate_mms(pg, s_wih[1], mix_rhs(hl0, land[0][par]), last=True)
                hl1 = ew(1, pg, t == 0, new_hl(1))

                # ---- slot 1: exchange h1(t); window: L2(t) ----
                xchg(1, t, hl1)
                pg = preissue(2, t, None if t == 0 else mix_rhs(hlp[2], land[2][prv]))
                gate_mms(pg, s_wih[2], mix_rhs(hl1, land[1][par]), last=True)
                hl2 = ew(2, pg, t == 0, new_hl(2))

                # ---- slot 2: exchange h2(t); window: dec -> enc -> L0(t+1) ----
                xchg(2, t, hl2)
                if not last_step:
                    pg = preissue(0, t + 1, mix_rhs(hl0, land[0][par]))
                rT = rtp.tile([P, KT_H, BG], BF16, tag="rT", name="rT")
                dec_pair(s_wd1, mix_rhs(hl2, land[2][par]), s_bd1, rT)
                if last_step:
                    outwrite(t, nat_rhs(rT))
                    break
                encT = encp.tile([P, KT_H, BG], BF16, tag="enc", name="enc")
                dec_pair(s_mm, nat_rhs(rT), s_cm, encT)
                gate_mms(pg, s_wih[0], nat_rhs(encT), last=True)
                hlp = [hl0, hl1, hl2]
                hl0 = ew(0, pg, False, new_hl(0))
                r_mix_prev = nat_rhs(rT)

            if comm == "rdma":
                # final sends drained before the NEFF exits
                with tc.tile_critical(no_gpsimd_drain=True):
                    nc.gpsimd.wait_ge(local_sem, 48 * xcnt[0])

    nc.compile()
    return nc


_CACHE = {}


def _get_program(timesteps, comm="rdma"):
    key = (timesteps, comm)
    if key not in _CACHE:
        _CACHE[key] = build_program(timesteps, comm)
    return _CACHE[key]


def _prep_inputs(x, We, be, W_ih, W_hh, b_ih, b_hh, Wd1, bd1, Wd2, bd2):
    """Host-side layout: shard/permute weights per core, fold biases."""
    f = np.float32
    bf = ml_dtypes.bfloat16
    x, We, be = np.asarray(x, f), np.asarray(We, f), np.asarray(be, f)
    W_ih, W_hh = np.asarray(W_ih, f), np.asarray(W_hh, f)
    b_ih, b_hh = np.asarray(b_ih, f), np.asarray(b_hh, f)
    Wd1, bd1 = np.asarray(Wd1, f), np.asarray(bd1, f)
    Wd2, bd2 = np.asarray(Wd2, f), np.asarray(bd2, f)

    enc0T = np.ascontiguousarray(np.maximum(x @ We.T + be, 0.0).T)  # [H, B]
    M = We @ Wd2                      # [H, H]; folds Wd2 then We (no relu between)
    cm = We @ bd2 + be                # [H]

    in_maps = []
    for k in range(NCORES):
        g, j = GROUP_OF[k], SL[k]
        # gate rows for my slice, order i, g, f, o
        rows = np.concatenate(
            [np.arange(G * H + j * SH, G * H + (j + 1) * SH) for G in (0, 2, 1, 3)]
        )
        # column permutation: k-blocks = [own, ^1, ^6, ^7] feature slices;
        # matches the landed-slot order of every exchanged quantity (h, r, enc)
        cols = np.concatenate(
            [np.arange(SL[k ^ d] * SH, (SL[k ^ d] + 1) * SH) for d in (0,) + DLOG]
        )
        ownd = np.arange(j * P, (j + 1) * P)         # my D_IN output slice
        m = {
            # decoder full (redundant per group); k columns XOR-permuted
            "wd1": np.ascontiguousarray(Wd1[:, cols].T).astype(bf),
            "mmat": np.ascontiguousarray(M.T).astype(bf),
            "wd2": np.ascontiguousarray(Wd2[ownd].T).astype(bf),
            "bd1r": bd1.reshape(1, H).astype(bf),
            "cmr": cm.reshape(1, H).astype(bf),
            "enc0": np.ascontiguousarray(enc0T[:, g * BG:(g + 1) * BG]).astype(bf),
        }
        for l in range(L):
            wih_l = W_ih[l][rows, :]
            if l > 0:
                wih_l = wih_l[:, cols]
            m[f"wih{l}"] = np.ascontiguousarray(wih_l.T).astype(bf)
            m[f"whh{l}"] = np.ascontiguousarray(W_hh[l][rows][:, cols].T).astype(bf)
            bsum = (b_ih[l] + b_hh[l])[rows]
            m[f"bg{l}"] = bsum.reshape(1, 4 * SH).astype(bf)
        in_maps.append(m)
    return in_maps, bd2


def kernel(x, We, be, W_ih, W_hh, b_ih, b_hh, Wd1, bd1, Wd2, bd2, timesteps, **run_kw):
    tsteps = int(timesteps)
    nc = _get_program(tsteps)
    in_maps, bd2_np = _prep_inputs(x, We, be, W_ih, W_hh, b_ih, b_hh, Wd1, bd1, Wd2, bd2)
    res = run_bass_kernel_spmd(nc, in_maps, core_ids=list(range(NCORES)), **run_kw)
    kernel.last_results = res
    out = np.empty((B, tsteps, D_IN), np.float32)
    for k in range(NCORES):
        g, j = GROUP_OF[k], SL[k]
        out[g * BG:(g + 1) * BG, :, j * P:(j + 1) * P] = np.asarray(
            res.results[k]["out"], np.float32)
    return out + bd2_np[None, None, :]
